# revision 8
# baseline (speedup 1.0000x reference)
"""ConvCNP encoder kernel for 8x TRN2 NeuronCores.

Math: the reference computes, for a 128x128 uniform grid g=(xs[i], ys[j]) and
n=8192 data points X (2-D) with values psi(Y) = [1, Y0, Y1]:

    Gram[g, x] = exp(-0.5*||g - X[x]||^2)
    fm = Gram @ psi                  # (G, 3); column 0 == row-sum (denominator)
    out[c, j, i] = fm[(i, j), c], with c=1,2 normalized by column 0.

The squared distance is separable over the grid axes:

    Gram[(i,j), x] = A[i, x] * B[j, x]
      A[i, x] = exp(-0.5*(xs[i] - X0[x])^2)     B[j, x] = exp(-0.5*(ys[j] - X1[x])^2)

so, with Bc = B * psi_c (row-wise):  fm[(i,j), c] = sum_x Bc[j, x] * A[i, x].

Sharding: the CONTRACTION axis (the 8192 points) is split across the 8 cores
- 1024 points (8 chunks of 128) per core; grid replicated. Each core computes
its partial fm over its point set:

    acc[i, (c,j)] = sum over 8 x-chunks of  AT_k^T @ BfT_k      (PE, PSUM accum)
      AT_k  = exp(-0.5*(xs[i] - X0[x])^2)   in SBUF layout [x_part=128, i=128]
      BfT_k = [B | B*Y0 | B*Y1]             in SBUF layout [x_part=128, 384]

and the host sums the 8 partial [128, 384] blocks (the unshard step for
contraction sharding), then normalizes c=1,2 by c=0.

Engine split per core (balancing DVE / ACT / PE):
  - AT's exponent comes from the PE as a K=3 outer product
        T[x,i] = X0[x]*xs[i] - 0.5*X0[x]^2 - 0.5*xs[i]^2   (PSUM f32)
    followed by a striped ACT Exp -> fp16.  This keeps the big [x, i] sqdiff
    off the DVE entirely.
  - BfT's B part is a fused custom DVE op sq(Src0 - Src1) + ACT Exp.
  - The B*Yc muls run on the DVE at the 2x packed 16-bit rate: Y is shipped
    8x-replicated ([x, k, c, jl] with jl=8 packed) so ALL mul operands have a
    stride-1 16-bit last dim - a stride-0 broadcast in the last dim would
    drop the DVE to the 1x fallback path.
"""

import numpy as np
from contextlib import ExitStack

N_AXIS = 128          # grid points per axis
NPTS = 8192           # data points
NCORES = 8
CPTS = NPTS // NCORES  # 1024 points per core
NCHUNK = CPTS // 128   # 8 contraction chunks of 128
GRID_LO, GRID_HI = -2.0, 2.0

_CACHE = {}


def _register_sqdiff():
    """Register a fused (a-b)^2 custom DVE op (idempotent)."""
    from concourse import dve_ops
    from concourse.dve_spec import Spec, Src0, Src1, sq, lower
    from concourse.dve_uop import DveOpSpec

    name = "TENSOR_SQDIFF_X"
    for op in dve_ops.OPS:
        if op.name == name:
            return op
    spec = Spec(
        body=sq(Src0 - Src1),
        reference=lambda in0, in1, s0, s1, imm2: (in0.astype(np.float32) - in1) ** 2,
    )
    opcode = max(dve_ops._SUB_OPCODE_FOR_NAME.values()) + 1
    assert opcode < 0x20
    dve_ops._SUB_OPCODE_FOR_NAME[name] = opcode
    shas = {}
    for ver in ("v3", "v4"):
        s = DveOpSpec(name=name, opcode=opcode, uops=lower(spec, ver=ver), rd1_en=True)
        shas[ver] = s.sha(ver)
    op = dve_ops.DveOp(name, spec, subdim=False, uops_sha=shas)
    dve_ops.OPS.append(op)
    dve_ops.CUSTOM_DVE_SPECS[name] = spec
    return op


def _build_program():
    import concourse.bacc as bacc
    import concourse.mybir as mybir
    import concourse.tile as tile

    sqdiff = _register_sqdiff()

    f32 = mybir.dt.float32
    f16 = mybir.dt.float16
    nc = bacc.Bacc("TRN2", target_bir_lowering=False, debug=False, num_devices=NCORES,
                   enable_partition_id=False, monotonic_sem_count=0)

    # Packed inputs (one DMA per queue, all issued immediately):
    #   ac [128, 136] f32: ysb(0:128) | x1t(128:136)           (sync queue)
    #   y8 [128, 128] f16: Y 8x-replicated, col = k*16+c*8+jl  (pool queue)
    #   pm [4, 1152]  f16: outer-product operands              (act queue)
    #        rows 0:3, cols k*128+x : [X0 | -0.5*X0^2 | 1] chunk k (stationary)
    #        rows 0:3, cols 1024:1152: [xs; 1; -0.5*xs^2]         (moving)
    ac = nc.dram_tensor("ac", [128, 136], f32, kind="ExternalInput")
    y8 = nc.dram_tensor("y8", [128, 128], f16, kind="ExternalInput")
    pm = nc.dram_tensor("pm", [4, 1152], f16, kind="ExternalInput")
    out = nc.dram_tensor("out", [128, 384], f32, kind="ExternalOutput")

    with tile.TileContext(nc) as tc, ExitStack() as ctx:
        singles = ctx.enter_context(tc.tile_pool(name="singles", bufs=1))
        psum = ctx.enter_context(tc.tile_pool(name="psum", bufs=1, space="PSUM"))

        # ac feeds the first DVE op - issue it from the scalar engine, which
        # exits the NEFF preamble earliest; pm (needed ~500ns later by the PE)
        # goes on the sync queue.
        s_ac = singles.tile([128, 136], f32, tag="ac")
        nc.scalar.dma_start(s_ac[:, :], ac[:, :])
        s_y8 = singles.tile([128, 128], f16, tag="y8")
        nc.gpsimd.dma_start(s_y8[:, :], y8[:, :])
        s_pm = singles.tile([4, 1152], f16, tag="pm")
        nc.sync.dma_start(s_pm[:, :], pm[:, :])

        ysb = s_ac[:, 0:128]
        x1t = s_ac[:, 128:136]

        s_argB = singles.tile([128, NCHUNK, 128], f32, tag="argB")
        s_bf = singles.tile([128, NCHUNK, 384], f16, tag="bf")
        s_at = singles.tile([128, NCHUNK, 128], f16, tag="at")
        psA = psum.tile([128, NCHUNK, 128], f32, tag="psA")
        acc = psum.tile([128, 384], f32, tag="acc")
        ps_warm = psum.tile([128, 512], f32, tag="warm")

        stripes = [(0, 4), (4, 4)]

        # ---- PE: A-exponent outer products (chunk k -> PSUM) ----
        for k in range(NCHUNK):
            nc.tensor.matmul(
                psA[:, k, :],
                s_pm[0:3, k * 128:(k + 1) * 128],   # lhsT [3, 128x]
                s_pm[0:3, 1024:1152],               # rhs  [3, 128i]
                start=True, stop=True,
            )
        # Keep the PE busy through the gap before the main matmuls: the PE's
        # HAM clock gate only reaches 2.4 GHz after ~3.4us of sustained
        # activity, and the main matmul chain is the span-critical tail.
        for _ in range(2):
            nc.tensor.matmul(
                ps_warm[:, :],
                s_pm[0:3, 0:128],
                s_pm[0:3, 0:512],
                start=True, stop=True,
            )

        # ---- DVE argB / ACT exps / DVE muls, striped ----
        for (k0, w) in stripes:
            nc.vector._custom_dve(
                sqdiff,
                out=s_argB[:, k0:k0 + w, :],
                in0=ysb.unsqueeze(1).broadcast_to([128, w, 128]),
                in1=x1t[:, k0:k0 + w].unsqueeze(2).broadcast_to([128, w, 128]),
            )

        # y8 view [x, k, c, jl]
        y8v = s_y8[:, :].rearrange("p (k c jl) -> p k c jl", c=2, jl=8)

        def emit_expB(k0, w):
            nc.scalar.activation(
                s_bf[:, k0:k0 + w, 0:128], s_argB[:, k0:k0 + w, :],
                mybir.ActivationFunctionType.Exp, scale=-0.5,
            )

        def emit_expA(k0, w):
            nc.scalar.activation(
                s_at[:, k0:k0 + w, :], psA[:, k0:k0 + w, :],
                mybir.ActivationFunctionType.Exp,
            )

        def emit_muls(k0, w):
            # bf[:, k, 128+c*128+j] = B[x,k,j] * Y_c[x,k]; j = jh*8+jl.
            # All operands keep a packed 16-bit last dim (jl) for DVE 2x.
            for c in range(2):
                nc.vector.tensor_tensor(
                    s_bf[:, k0:k0 + w, 128 + c * 128:256 + c * 128]
                        .rearrange("p w (jh jl) -> p w jh jl", jl=8),
                    s_bf[:, k0:k0 + w, 0:128]
                        .rearrange("p w (jh jl) -> p w jh jl", jl=8),
                    y8v[:, k0:k0 + w, c, :].unsqueeze(2)
                        .broadcast_to([128, w, 16, 8]),
                    mybir.AluOpType.mult,
                )

        emit_expB(*stripes[0])
        emit_expA(*stripes[0])
        emit_muls(*stripes[0])
        emit_expB(*stripes[1])
        emit_muls(*stripes[1])
        emit_expA(*stripes[1])

        # ---- PE: 8 accumulating matmuls acc[i, (c,j)] ----
        for k in range(NCHUNK):
            nc.tensor.matmul(
                acc[:, :],
                s_at[:, k, :],   # stationary lhsT: [128x, 128i] fp16
                s_bf[:, k, :],   # moving rhs: [128x, 384] fp16
                start=(k == 0),
                stop=(k == NCHUNK - 1),
            )

        # ---- epilogue: PSUM -> SBUF -> HBM (partial sums; host reduces) ----
        s_out = singles.tile([128, 384], f32, tag="outt")
        nc.vector.tensor_copy(s_out[:, :], acc[:, :])
        nc.sync.dma_start(out[:, :], s_out[:, :])

    nc.finalize()
    return nc


def _get_program():
    if "nc" not in _CACHE:
        _CACHE["nc"] = _build_program()
    return _CACHE["nc"]


def _host_inputs(X, Y):
    """Build the per-core input maps (layout prep only)."""
    X = np.ascontiguousarray(np.asarray(X, dtype=np.float32))
    Y = np.ascontiguousarray(np.asarray(Y, dtype=np.float32))
    xs = np.linspace(GRID_LO, GRID_HI, N_AXIS, dtype=np.float32)
    ys = np.linspace(GRID_LO, GRID_HI, N_AXIS, dtype=np.float32)

    in_maps = []
    for m in range(NCORES):
        sl = slice(m * CPTS, (m + 1) * CPTS)
        x0 = X[sl, 0]
        acm = np.empty((128, 136), np.float32)
        acm[:, 0:128] = ys[None, :]
        acm[:, 128:136] = X[sl, 1].reshape(NCHUNK, 128).T
        # y8[x, k*16 + c*8 + jl] = Y[chunk k, point x, c]
        y8m = np.empty((128, NCHUNK, 2, 8), np.float16)
        y8m[:, :, 0, :] = Y[sl, 0].reshape(NCHUNK, 128).T[:, :, None]
        y8m[:, :, 1, :] = Y[sl, 1].reshape(NCHUNK, 128).T[:, :, None]
        pmm = np.zeros((4, 1152), np.float16)
        pmm[0, 0:1024] = x0
        pmm[1, 0:1024] = -0.5 * x0.astype(np.float64) ** 2
        pmm[2, 0:1024] = 1.0
        pmm[0, 1024:1152] = xs
        pmm[1, 1024:1152] = 1.0
        pmm[2, 1024:1152] = -0.5 * xs.astype(np.float64) ** 2
        in_maps.append({"ac": acm, "y8": y8m.reshape(128, 128), "pm": pmm})
    return in_maps


def run_on_cores(X, Y, **spmd_kwargs):
    """Run the SPMD kernel; returns BassKernelResults."""
    from concourse.bass_utils import run_bass_kernel_spmd

    nc = _get_program()
    in_maps = _host_inputs(X, Y)
    res = run_bass_kernel_spmd(nc, in_maps, core_ids=list(range(NCORES)),
                               **spmd_kwargs)
    return res


def kernel(X, Y):
    res = run_on_cores(X, Y)
    # Sum the per-core partial contractions (contraction-axis unshard).
    fm = np.zeros((128, 384), dtype=np.float32)
    for r in res.results:
        fm += r["out"]
    full = np.empty((3, N_AXIS, N_AXIS), dtype=np.float32)
    den = fm[:, 0:128]
    full[0] = den.T
    full[1] = (fm[:, 128:256] / den).T
    full[2] = (fm[:, 256:384] / den).T
    return full


# revision 10
# speedup vs baseline: 1.0186x; 1.0186x over previous
"""ConvCNP encoder kernel for 8x TRN2 NeuronCores.

Math: the reference computes, for a 128x128 uniform grid g=(xs[i], ys[j]) and
n=8192 data points X (2-D) with values psi(Y) = [1, Y0, Y1]:

    Gram[g, x] = exp(-0.5*||g - X[x]||^2)
    fm = Gram @ psi                  # (G, 3); column 0 == row-sum (denominator)
    out[c, j, i] = fm[(i, j), c], with c=1,2 normalized by column 0.

The squared distance is separable over the grid axes:

    Gram[(i,j), x] = A[i, x] * B[j, x]
      A[i, x] = exp(-0.5*(xs[i] - X0[x])^2)     B[j, x] = exp(-0.5*(ys[j] - X1[x])^2)

so, with Bc = B * psi_c (row-wise):  fm[(i,j), c] = sum_x Bc[j, x] * A[i, x].

Sharding: the CONTRACTION axis (the 8192 points) is split across the 8 cores
- 1024 points (8 chunks of 128) per core; grid replicated. Each core computes
its partial fm over its point set:

    acc[i, (c,j)] = sum over 8 x-chunks of  AT_k^T @ BfT_k      (PE, PSUM accum)
      AT_k  = exp(-0.5*(xs[i] - X0[x])^2)   in SBUF layout [x_part=128, i=128]
      BfT_k = [B | B*Y0 | B*Y1]             in SBUF layout [x_part=128, 384]

and the host sums the 8 partial [128, 384] blocks (the unshard step for
contraction sharding), then normalizes c=1,2 by c=0.

Engine split per core (balancing DVE / ACT / PE):
  - AT's exponent comes from the PE as a K=3 outer product
        T[x,i] = X0[x]*xs[i] - 0.5*X0[x]^2 - 0.5*xs[i]^2   (PSUM f32)
    followed by a striped ACT Exp -> fp16.  This keeps the big [x, i] sqdiff
    off the DVE entirely.
  - BfT's B part is a fused custom DVE op sq(Src0 - Src1) + ACT Exp.
  - The B*Yc muls run on the DVE at the 2x packed 16-bit rate: Y is shipped
    8x-replicated ([x, k, c, jl] with jl=8 packed) so ALL mul operands have a
    stride-1 16-bit last dim - a stride-0 broadcast in the last dim would
    drop the DVE to the 1x fallback path.
"""

import numpy as np
from contextlib import ExitStack

N_AXIS = 128          # grid points per axis
NPTS = 8192           # data points
NCORES = 8
CPTS = NPTS // NCORES  # 1024 points per core
NCHUNK = CPTS // 128   # 8 contraction chunks of 128
GRID_LO, GRID_HI = -2.0, 2.0

_CACHE = {}


def _register_sqdiff():
    """Register a fused (a-b)^2 custom DVE op (idempotent)."""
    from concourse import dve_ops
    from concourse.dve_spec import Spec, Src0, Src1, sq, lower
    from concourse.dve_uop import DveOpSpec

    name = "TENSOR_SQDIFF_X"
    for op in dve_ops.OPS:
        if op.name == name:
            return op
    spec = Spec(
        body=sq(Src0 - Src1),
        reference=lambda in0, in1, s0, s1, imm2: (in0.astype(np.float32) - in1) ** 2,
    )
    opcode = max(dve_ops._SUB_OPCODE_FOR_NAME.values()) + 1
    assert opcode < 0x20
    dve_ops._SUB_OPCODE_FOR_NAME[name] = opcode
    shas = {}
    for ver in ("v3", "v4"):
        s = DveOpSpec(name=name, opcode=opcode, uops=lower(spec, ver=ver), rd1_en=True)
        shas[ver] = s.sha(ver)
    op = dve_ops.DveOp(name, spec, subdim=False, uops_sha=shas)
    dve_ops.OPS.append(op)
    dve_ops.CUSTOM_DVE_SPECS[name] = spec
    return op


def _build_program():
    import concourse.bacc as bacc
    import concourse.mybir as mybir
    import concourse.tile as tile

    sqdiff = _register_sqdiff()

    f32 = mybir.dt.float32
    f16 = mybir.dt.float16
    nc = bacc.Bacc("TRN2", target_bir_lowering=False, debug=False, num_devices=NCORES,
                   enable_partition_id=False, monotonic_sem_count=0)

    # Packed inputs (one DMA per queue, all issued immediately):
    #   ac [128, 136] f32: ysb(0:128) | x1t(128:136)           (sync queue)
    #   y8 [128, 128] f16: Y 8x-replicated, col = k*16+c*8+jl  (pool queue)
    #   pm [4, 1152]  f16: outer-product operands              (act queue)
    #        rows 0:3, cols k*128+x : [X0 | -0.5*X0^2 | 1] chunk k (stationary)
    #        rows 0:3, cols 1024:1152: [xs; 1; -0.5*xs^2]         (moving)
    ac = nc.dram_tensor("ac", [128, 136], f32, kind="ExternalInput")
    y8 = nc.dram_tensor("y8", [128, 128], f16, kind="ExternalInput")
    pm = nc.dram_tensor("pm", [4, 1152], f16, kind="ExternalInput")
    out = nc.dram_tensor("out", [128, 384], f32, kind="ExternalOutput")

    with tile.TileContext(nc) as tc, ExitStack() as ctx:
        singles = ctx.enter_context(tc.tile_pool(name="singles", bufs=1))
        psum = ctx.enter_context(tc.tile_pool(name="psum", bufs=1, space="PSUM"))

        # Both latency-critical inputs go on the sync HWDGE queue (measured
        # fastest delivery); ac first since it feeds the first DVE op. The
        # scalar queue stalls behind the ACT table load, gpsimd's SWDGE is
        # slow - y8 (not needed until the muls ~2us later) rides there.
        s_ac = singles.tile([128, 136], f32, tag="ac")
        nc.sync.dma_start(s_ac[:, :], ac[:, :])
        s_pm = singles.tile([4, 1152], f16, tag="pm")
        nc.sync.dma_start(s_pm[:, :], pm[:, :])
        s_y8 = singles.tile([128, 128], f16, tag="y8")
        nc.gpsimd.dma_start(s_y8[:, :], y8[:, :])

        ysb = s_ac[:, 0:128]
        x1t = s_ac[:, 128:136]

        s_argB = singles.tile([128, NCHUNK, 128], f32, tag="argB")
        s_bf = singles.tile([128, NCHUNK, 384], f16, tag="bf")
        s_at = singles.tile([128, NCHUNK, 128], f16, tag="at")
        psA = psum.tile([128, NCHUNK, 128], f32, tag="psA")
        acc = psum.tile([128, 384], f32, tag="acc")
        ps_warm = psum.tile([128, 512], f32, tag="warm")

        stripes = [(0, 4), (4, 4)]

        # ---- PE: A-exponent outer products (chunk k -> PSUM) ----
        for k in range(NCHUNK):
            nc.tensor.matmul(
                psA[:, k, :],
                s_pm[0:3, k * 128:(k + 1) * 128],   # lhsT [3, 128x]
                s_pm[0:3, 1024:1152],               # rhs  [3, 128i]
                start=True, stop=True,
            )
        # Keep the PE busy through the gap before the main matmuls: the PE's
        # HAM clock gate only reaches 2.4 GHz after ~3.4us of sustained
        # activity, and the main matmul chain is the span-critical tail.
        for _ in range(5):
            nc.tensor.matmul(
                ps_warm[:, :],
                s_pm[0:3, 0:128],
                s_pm[0:3, 0:512],
                start=True, stop=True,
            )

        # ---- DVE argB / ACT exps / DVE muls, striped ----
        for (k0, w) in stripes:
            nc.vector._custom_dve(
                sqdiff,
                out=s_argB[:, k0:k0 + w, :],
                in0=ysb.unsqueeze(1).broadcast_to([128, w, 128]),
                in1=x1t[:, k0:k0 + w].unsqueeze(2).broadcast_to([128, w, 128]),
            )

        # y8 view [x, k, c, jl]
        y8v = s_y8[:, :].rearrange("p (k c jl) -> p k c jl", c=2, jl=8)

        def emit_expB(k0, w):
            nc.scalar.activation(
                s_bf[:, k0:k0 + w, 0:128], s_argB[:, k0:k0 + w, :],
                mybir.ActivationFunctionType.Exp, scale=-0.5,
            )

        def emit_expA(k0, w):
            nc.scalar.activation(
                s_at[:, k0:k0 + w, :], psA[:, k0:k0 + w, :],
                mybir.ActivationFunctionType.Exp,
            )

        def emit_muls(k0, w):
            # bf[:, k, 128+c*128+j] = B[x,k,j] * Y_c[x,k]; j = jh*8+jl.
            # All operands keep a packed 16-bit last dim (jl) for DVE 2x.
            for c in range(2):
                nc.vector.tensor_tensor(
                    s_bf[:, k0:k0 + w, 128 + c * 128:256 + c * 128]
                        .rearrange("p w (jh jl) -> p w jh jl", jl=8),
                    s_bf[:, k0:k0 + w, 0:128]
                        .rearrange("p w (jh jl) -> p w jh jl", jl=8),
                    y8v[:, k0:k0 + w, c, :].unsqueeze(2)
                        .broadcast_to([128, w, 16, 8]),
                    mybir.AluOpType.mult,
                )

        emit_expB(*stripes[0])
        emit_expA(*stripes[0])
        emit_muls(*stripes[0])
        emit_expB(*stripes[1])
        emit_muls(*stripes[1])
        emit_expA(*stripes[1])

        # ---- PE: 8 accumulating matmuls acc[i, (c,j)] ----
        for k in range(NCHUNK):
            nc.tensor.matmul(
                acc[:, :],
                s_at[:, k, :],   # stationary lhsT: [128x, 128i] fp16
                s_bf[:, k, :],   # moving rhs: [128x, 384] fp16
                start=(k == 0),
                stop=(k == NCHUNK - 1),
            )

        # ---- epilogue: PSUM -> SBUF -> HBM (partial sums; host reduces) ----
        s_out = singles.tile([128, 384], f32, tag="outt")
        nc.vector.tensor_copy(s_out[:, :], acc[:, :])
        nc.sync.dma_start(out[:, :], s_out[:, :])

    nc.finalize()
    return nc


def _get_program():
    if "nc" not in _CACHE:
        _CACHE["nc"] = _build_program()
    return _CACHE["nc"]


def _host_inputs(X, Y):
    """Build the per-core input maps (layout prep only)."""
    X = np.ascontiguousarray(np.asarray(X, dtype=np.float32))
    Y = np.ascontiguousarray(np.asarray(Y, dtype=np.float32))
    xs = np.linspace(GRID_LO, GRID_HI, N_AXIS, dtype=np.float32)
    ys = np.linspace(GRID_LO, GRID_HI, N_AXIS, dtype=np.float32)

    in_maps = []
    for m in range(NCORES):
        sl = slice(m * CPTS, (m + 1) * CPTS)
        x0 = X[sl, 0]
        acm = np.empty((128, 136), np.float32)
        acm[:, 0:128] = ys[None, :]
        acm[:, 128:136] = X[sl, 1].reshape(NCHUNK, 128).T
        # y8[x, k*16 + c*8 + jl] = Y[chunk k, point x, c]
        y8m = np.empty((128, NCHUNK, 2, 8), np.float16)
        y8m[:, :, 0, :] = Y[sl, 0].reshape(NCHUNK, 128).T[:, :, None]
        y8m[:, :, 1, :] = Y[sl, 1].reshape(NCHUNK, 128).T[:, :, None]
        pmm = np.zeros((4, 1152), np.float16)
        pmm[0, 0:1024] = x0
        pmm[1, 0:1024] = -0.5 * x0.astype(np.float64) ** 2
        pmm[2, 0:1024] = 1.0
        pmm[0, 1024:1152] = xs
        pmm[1, 1024:1152] = 1.0
        pmm[2, 1024:1152] = -0.5 * xs.astype(np.float64) ** 2
        in_maps.append({"ac": acm, "y8": y8m.reshape(128, 128), "pm": pmm})
    return in_maps


def run_on_cores(X, Y, **spmd_kwargs):
    """Run the SPMD kernel; returns BassKernelResults."""
    from concourse.bass_utils import run_bass_kernel_spmd

    nc = _get_program()
    in_maps = _host_inputs(X, Y)
    res = run_bass_kernel_spmd(nc, in_maps, core_ids=list(range(NCORES)),
                               **spmd_kwargs)
    return res


def kernel(X, Y):
    res = run_on_cores(X, Y)
    # Sum the per-core partial contractions (contraction-axis unshard).
    fm = np.zeros((128, 384), dtype=np.float32)
    for r in res.results:
        fm += r["out"]
    full = np.empty((3, N_AXIS, N_AXIS), dtype=np.float32)
    den = fm[:, 0:128]
    full[0] = den.T
    full[1] = (fm[:, 128:256] / den).T
    full[2] = (fm[:, 256:384] / den).T
    return full


# revision 14
# speedup vs baseline: 1.0958x; 1.0758x over previous
"""ConvCNP encoder kernel for 8x TRN2 NeuronCores.

Math: the reference computes, for a 128x128 uniform grid g=(xs[i], ys[j]) and
n=8192 data points X (2-D) with values psi(Y) = [1, Y0, Y1]:

    Gram[g, x] = exp(-0.5*||g - X[x]||^2)
    fm = Gram @ psi                  # (G, 3); column 0 == row-sum (denominator)
    out[c, j, i] = fm[(i, j), c], with c=1,2 normalized by column 0.

The squared distance is separable over the grid axes:

    Gram[(i,j), x] = A[i, x] * B[j, x]
      A[i, x] = exp(-0.5*(xs[i] - X0[x])^2)     B[j, x] = exp(-0.5*(ys[j] - X1[x])^2)

so, with Bc = B * psi_c (row-wise):  fm[(i,j), c] = sum_x Bc[j, x] * A[i, x].

Sharding: the CONTRACTION axis (the 8192 points) is split across the 8 cores
- 1024 points (8 chunks of 128) per core; grid replicated. Each core computes
its partial fm over its point set:

    acc[i, (c,j)] = sum over 8 x-chunks of  AT_k^T @ BfT_k      (PE, PSUM accum)
      AT_k  = exp(-0.5*(xs[i] - X0[x])^2)   in SBUF layout [x_part=128, i=128]
      BfT_k = [B | B*Y0 | B*Y1]             in SBUF layout [x_part=128, 384]

and the host sums the 8 partial [128, 384] blocks (the unshard step for
contraction sharding), then normalizes c=1,2 by c=0.

Engine split per core (balancing DVE / ACT / PE):
  - AT's exponent comes from the PE as a K=3 outer product
        T[x,i] = X0[x]*xs[i] - 0.5*X0[x]^2 - 0.5*xs[i]^2   (PSUM f32)
    followed by a striped ACT Exp -> fp16.  This keeps the big [x, i] sqdiff
    off the DVE entirely.
  - BfT's B part is a fused custom DVE op sq(Src0 - Src1) + ACT Exp.
  - The B*Yc muls run on the DVE at the 2x packed 16-bit rate: Y is shipped
    8x-replicated ([x, k, c, jl] with jl=8 packed) so ALL mul operands have a
    stride-1 16-bit last dim - a stride-0 broadcast in the last dim would
    drop the DVE to the 1x fallback path.
"""

import numpy as np
from contextlib import ExitStack

N_AXIS = 128          # grid points per axis
NPTS = 8192           # data points
NCORES = 8
CPTS = NPTS // NCORES  # 1024 points per core
NCHUNK = CPTS // 128   # 8 contraction chunks of 128
GRID_LO, GRID_HI = -2.0, 2.0

_CACHE = {}


def _register_sqdiff():
    """Register a fused (a-b)^2 custom DVE op (idempotent)."""
    from concourse import dve_ops
    from concourse.dve_spec import Spec, Src0, Src1, sq, lower
    from concourse.dve_uop import DveOpSpec

    name = "TENSOR_SQDIFF_X"
    for op in dve_ops.OPS:
        if op.name == name:
            return op
    spec = Spec(
        body=sq(Src0 - Src1),
        reference=lambda in0, in1, s0, s1, imm2: (in0.astype(np.float32) - in1) ** 2,
    )
    opcode = max(dve_ops._SUB_OPCODE_FOR_NAME.values()) + 1
    assert opcode < 0x20
    dve_ops._SUB_OPCODE_FOR_NAME[name] = opcode
    shas = {}
    for ver in ("v3", "v4"):
        s = DveOpSpec(name=name, opcode=opcode, uops=lower(spec, ver=ver), rd1_en=True)
        shas[ver] = s.sha(ver)
    op = dve_ops.DveOp(name, spec, subdim=False, uops_sha=shas)
    dve_ops.OPS.append(op)
    dve_ops.CUSTOM_DVE_SPECS[name] = spec
    return op


def _build_program():
    import concourse.bacc as bacc
    import concourse.mybir as mybir
    import concourse.tile as tile

    sqdiff = _register_sqdiff()

    f32 = mybir.dt.float32
    f16 = mybir.dt.float16
    nc = bacc.Bacc("TRN2", target_bir_lowering=False, debug=False, num_devices=NCORES,
                   enable_partition_id=False, monotonic_sem_count=0)

    # Packed inputs (one DMA per queue, all issued immediately):
    #   ac [128, 136] f32: ysb(0:128) | x1t(128:136)           (sync queue)
    #   y8 [128, 128] f16: Y 8x-replicated, col = k*16+c*8+jl  (pool queue)
    #   pm [4, 1152]  f16: outer-product operands              (act queue)
    #        rows 0:3, cols k*128+x : [X0 | -0.5*X0^2 | 1] chunk k (stationary)
    #        rows 0:3, cols 1024:1152: [xs; 1; -0.5*xs^2]         (moving)
    ac = nc.dram_tensor("ac", [128, 136], f32, kind="ExternalInput")
    y8 = nc.dram_tensor("y8", [128, 128], f16, kind="ExternalInput")
    pm = nc.dram_tensor("pm", [4, 1152], f16, kind="ExternalInput")
    out = nc.dram_tensor("out", [128, 384], f16, kind="ExternalOutput")

    with tile.TileContext(nc) as tc, ExitStack() as ctx:
        singles = ctx.enter_context(tc.tile_pool(name="singles", bufs=1))
        psum = ctx.enter_context(tc.tile_pool(name="psum", bufs=1, space="PSUM"))

        # Both latency-critical inputs go on the sync HWDGE queue (measured
        # fastest delivery); ac first since it feeds the first DVE op. The
        # scalar queue stalls behind the ACT table load, gpsimd's SWDGE is
        # slow - y8 (not needed until the muls ~2us later) rides there.
        s_ac = singles.tile([128, 136], f32, tag="ac")
        nc.sync.dma_start(s_ac[:, :], ac[:, :])
        s_pm = singles.tile([4, 1152], f16, tag="pm")
        nc.sync.dma_start(s_pm[:, :], pm[:, :])
        s_y8 = singles.tile([128, 128], f16, tag="y8")
        nc.gpsimd.dma_start(s_y8[:, :], y8[:, :])

        ysb = s_ac[:, 0:128]
        x1t = s_ac[:, 128:136]

        s_argB = singles.tile([128, NCHUNK, 128], f32, tag="argB")
        s_bf = singles.tile([128, NCHUNK, 384], f16, tag="bf")
        s_at = singles.tile([128, NCHUNK, 128], f16, tag="at")
        psA = psum.tile([128, NCHUNK, 128], f32, tag="psA")
        acc = psum.tile([128, 384], f32, tag="acc")

        stripes = [(0, 4), (4, 4)]

        # ---- PE: A-exponent outer products (chunk k -> PSUM) ----
        for k in range(NCHUNK):
            nc.tensor.matmul(
                psA[:, k, :],
                s_pm[0:3, k * 128:(k + 1) * 128],   # lhsT [3, 128x]
                s_pm[0:3, 1024:1152],               # rhs  [3, 128i]
                start=True, stop=True,
            )

        # ---- DVE argB / ACT exps / DVE muls, striped ----
        for (k0, w) in stripes:
            nc.vector._custom_dve(
                sqdiff,
                out=s_argB[:, k0:k0 + w, :],
                in0=ysb.unsqueeze(1).broadcast_to([128, w, 128]),
                in1=x1t[:, k0:k0 + w].unsqueeze(2).broadcast_to([128, w, 128]),
            )

        # y8 view [x, k, c, jl]
        y8v = s_y8[:, :].rearrange("p (k c jl) -> p k c jl", c=2, jl=8)

        def emit_expB(k0, w):
            nc.scalar.activation(
                s_bf[:, k0:k0 + w, 0:128], s_argB[:, k0:k0 + w, :],
                mybir.ActivationFunctionType.Exp, scale=-0.5,
            )

        def emit_expA(k0, w):
            nc.scalar.activation(
                s_at[:, k0:k0 + w, :], psA[:, k0:k0 + w, :],
                mybir.ActivationFunctionType.Exp,
            )

        def emit_muls(k0, w):
            # bf[:, k, 128+c*128+j] = B[x,k,j] * Y_c[x,k]; j = jh*8+jl.
            # All operands keep a packed 16-bit last dim (jl) for DVE 2x.
            for c in range(2):
                nc.vector.tensor_tensor(
                    s_bf[:, k0:k0 + w, 128 + c * 128:256 + c * 128]
                        .rearrange("p w (jh jl) -> p w jh jl", jl=8),
                    s_bf[:, k0:k0 + w, 0:128]
                        .rearrange("p w (jh jl) -> p w jh jl", jl=8),
                    y8v[:, k0:k0 + w, c, :].unsqueeze(2)
                        .broadcast_to([128, w, 16, 8]),
                    mybir.AluOpType.mult,
                )

        emit_expB(*stripes[0])
        emit_expA(*stripes[0])
        emit_muls(*stripes[0])
        emit_expB(*stripes[1])
        emit_muls(*stripes[1])
        emit_expA(*stripes[1])

        # ---- PE: 8 accumulating matmuls acc[i, (c,j)] ----
        for k in range(NCHUNK):
            nc.tensor.matmul(
                acc[:, :],
                s_at[:, k, :],   # stationary lhsT: [128x, 128i] fp16
                s_bf[:, k, :],   # moving rhs: [128x, 384] fp16
                start=(k == 0),
                stop=(k == NCHUNK - 1),
            )

        # ---- epilogue: PSUM -> SBUF (f16 partials halve the out DMA) ----
        s_out = singles.tile([128, 384], f16, tag="outt")
        nc.vector.tensor_copy(s_out[:, :], acc[:, :])
        nc.sync.dma_start(out[:, :], s_out[:, :])

    nc.finalize()
    return nc


def _get_program():
    if "nc" not in _CACHE:
        _CACHE["nc"] = _build_program()
    return _CACHE["nc"]


def _host_inputs(X, Y):
    """Build the per-core input maps (layout prep only)."""
    X = np.ascontiguousarray(np.asarray(X, dtype=np.float32))
    Y = np.ascontiguousarray(np.asarray(Y, dtype=np.float32))
    xs = np.linspace(GRID_LO, GRID_HI, N_AXIS, dtype=np.float32)
    ys = np.linspace(GRID_LO, GRID_HI, N_AXIS, dtype=np.float32)

    in_maps = []
    for m in range(NCORES):
        sl = slice(m * CPTS, (m + 1) * CPTS)
        x0 = X[sl, 0]
        acm = np.empty((128, 136), np.float32)
        acm[:, 0:128] = ys[None, :]
        acm[:, 128:136] = X[sl, 1].reshape(NCHUNK, 128).T
        # y8[x, k*16 + c*8 + jl] = Y[chunk k, point x, c]
        y8m = np.empty((128, NCHUNK, 2, 8), np.float16)
        y8m[:, :, 0, :] = Y[sl, 0].reshape(NCHUNK, 128).T[:, :, None]
        y8m[:, :, 1, :] = Y[sl, 1].reshape(NCHUNK, 128).T[:, :, None]
        pmm = np.zeros((4, 1152), np.float16)
        pmm[0, 0:1024] = x0
        pmm[1, 0:1024] = -0.5 * x0.astype(np.float64) ** 2
        pmm[2, 0:1024] = 1.0
        pmm[0, 1024:1152] = xs
        pmm[1, 1024:1152] = 1.0
        pmm[2, 1024:1152] = -0.5 * xs.astype(np.float64) ** 2
        in_maps.append({"ac": acm, "y8": y8m.reshape(128, 128), "pm": pmm})
    return in_maps


def run_on_cores(X, Y, **spmd_kwargs):
    """Run the SPMD kernel; returns BassKernelResults."""
    from concourse.bass_utils import run_bass_kernel_spmd

    nc = _get_program()
    in_maps = _host_inputs(X, Y)
    res = run_bass_kernel_spmd(nc, in_maps, core_ids=list(range(NCORES)),
                               **spmd_kwargs)
    return res


def kernel(X, Y):
    res = run_on_cores(X, Y)
    # Sum the per-core partial contractions (contraction-axis unshard).
    fm = np.zeros((128, 384), dtype=np.float32)
    for r in res.results:
        fm += r["out"].astype(np.float32)
    full = np.empty((3, N_AXIS, N_AXIS), dtype=np.float32)
    den = fm[:, 0:128]
    full[0] = den.T
    full[1] = (fm[:, 128:256] / den).T
    full[2] = (fm[:, 256:384] / den).T
    return full
